# revision 1
# baseline (speedup 1.0000x reference)
"""Multi-head self-attention (B=2, T=2048, C=1024, H=16, RoPE, causal) on 8 trn2 cores.

Sharding: data-parallel over batch (2) x tensor-parallel over head groups (4).
Core c handles batch c//4, heads (c%4)*4 .. +3.  Each core computes its
4 heads' attention output and a partial out-projection (contraction over its
256 head-dims); the host sums the 4 partials per batch.

Layout strategy per core:
  - x^T tiles (C on partitions) built via PE transposes.
  - QKV projection emits q^T/k^T with per-head [evens(32); odds(32)] dim
    order (host-permuted w columns), so RoPE is full-width DVE math in a
    [QE_all(128); QO_all(128)] layout, then SBUF->SBUF DMAs merge into
    per-head-contiguous q^T/k^T tiles (two heads per 128-partition tile).
  - Scores computed transposed (S^T[k, q]) so no P-transposes are needed:
    softmax denominator comes free from a ones-column in the O^T stationary.
  - fp32r matmuls throughout (full-speed fp32 mode on trn2).
"""
import sys
import math

sys.path.insert(0, "/opt/trn_rl_repo")

import numpy as np

B, T, C, H, D = 2, 2048, 1024, 16, 64
HG = H // 4            # 4 heads per core
NCORES = 8
NKC = C // 128         # 8 contraction chunks
NQTR = T // 512        # 4 t-quarters
NKT = T // 128         # 16 k-tiles
ROPE_BASE = 10000.0

_BUILT = None


# ---------------------------------------------------------------------------
# Toolchain workaround: this walrus build accepts at most ONE semaphore wait
# per instruction.  Tile's exit drain carries one wait per outstanding proc,
# and stage-1B can attach several waits to compute/DMA instructions.  We
# (a) replace the exit drain with a chain of single-wait drains, and
# (b) post-process the module, hoisting extra waits onto same-engine nops.
# ---------------------------------------------------------------------------

def _apply_tile_patch():
    import bass_rust
    import concourse.tile as tile
    from concourse.vector_clock import ScopedClock

    def _patched_drain_and_barrier(self, tick_clock, wait_clock):
        nc = self.nc
        probe = nc.sync.drain()
        wait_clock.add_sem_waits(probe.ins, ScopedClock({None: tick_clock.global_clock}))
        si = probe.ins.sync_info
        waits = list(si.on_wait) if si is not None else []
        probe.ins.sync_info = None
        name2sem = {s.name: s for s in wait_clock.sems.allocated().values()}
        for w in waits:
            d = nc.sync.drain()
            bass_rust.wait_op(d.ins, name2sem[w.ant_name], w.wait_value, "sem-ge", False)
        nc.all_engine_barrier()
        popped = nc._tile_sem_poison_stack.pop()
        assert popped is self._sem_poison
        nc.clear_and_free_semaphores(list(self.sems.allocated().values()))
        nc.all_engine_barrier()

    tile.TileContext._drain_and_barrier = _patched_drain_and_barrier


def _split_multi_waits(nc):
    import bass_rust
    import concourse.mybir as mybir

    ctr = 0
    for fn in nc.m.functions:
        for blk in fn.blocks:
            il = blk.instructions
            new = []
            changed = False
            for inst in il:
                si = inst.sync_info
                waits = list(si.on_wait) if si is not None else []
                if len(waits) > 1:
                    changed = True
                    for w in waits[:-1]:
                        nop = mybir.InstNoOp(name=f"I-waitsplit-{ctr}", ins=[], outs=[])
                        ctr += 1
                        nop.engine = inst.engine
                        nop.sync_info = bass_rust.SyncInfo(on_wait=[w], on_update=[])
                        new.append(nop)
                    inst.sync_info = bass_rust.SyncInfo(
                        on_wait=[waits[-1]], on_update=list(si.on_update)
                    )
                new.append(inst)
            if changed:
                blk.instructions = new


# ---------------------------------------------------------------------------
# Kernel builder (per-core program; identical on all 8 cores)
# ---------------------------------------------------------------------------

def build_nc(split_waits=True, loop_iters=None, phases=(1, 2)):
    _apply_tile_patch()
    import concourse.bass as bass
    import concourse.tile as tile
    import concourse.mybir as mybir
    from concourse.masks import make_identity
    from contextlib import nullcontext

    dt = mybir.dt
    f32, f32r = dt.float32, dt.float32r
    Exp = mybir.ActivationFunctionType.Exp
    MUL, SUB, ADD, DIV = (mybir.AluOpType.mult, mybir.AluOpType.subtract,
                          mybir.AluOpType.add, mybir.AluOpType.divide)

    nc = bass.Bass()
    x_d = nc.dram_tensor("x", [T, C], f32, kind="ExternalInput")
    w_d = nc.dram_tensor("w", [C, 768], f32, kind="ExternalInput")
    wo_d = nc.dram_tensor("wo", [256, C], f32, kind="ExternalInput")
    cs_d = nc.dram_tensor("cs", [128, T], f32, kind="ExternalInput")
    sn_d = nc.dram_tensor("sn", [128, T], f32, kind="ExternalInput")
    tri_d = nc.dram_tensor("tri", [128, 128], f32, kind="ExternalInput")
    y_d = nc.dram_tensor("y", [T, C], f32, kind="ExternalOutput")

    with tile.TileContext(nc) as tc:
      loop_cm = (tc.For_i(0, loop_iters, 1,
                          hint_engines=(mybir.EngineType.PE, mybir.EngineType.Activation,
                                        mybir.EngineType.DVE, mybir.EngineType.SP,
                                        mybir.EngineType.Pool))
                 if loop_iters else nullcontext())
      with loop_cm:
        with (
            # ---- persistent pools (live across all phases)
            tc.tile_pool(name="persist", bufs=1) as persist,
            tc.tile_pool(name="qkT", bufs=1) as qkT_pool,
            tc.tile_pool(name="asb", bufs=1) as asb_pool,
        ):
            # per-head-contiguous rotated q^T/k^T: tile [128, T] = 2 heads
            qT = [qkT_pool.tile([128, T], f32r, tag=f"qT{i}", name=f"qT{i}") for i in range(2)]
            kT = [qkT_pool.tile([128, T], f32r, tag=f"kT{i}", name=f"kT{i}") for i in range(2)]
            # v in (t, d) layout + ones column per head slot: [128, kt, 4*65]
            v_sb = persist.tile([128, NKT, 4 * 65], f32r, tag="v")
            wo_sb = persist.tile([128, 2, C], f32r, tag="wo")
            tri_sb = persist.tile([128, 128], f32, tag="tri")
            cs_sb = persist.tile([128, T], f32, tag="cs")
            sn_sb = persist.tile([128, T], f32, tag="sn")
            a_sb = [asb_pool.tile([128, T], f32r, tag=f"a{i}", name=f"a{i}") for i in range(2)]

            # ones columns of v (col 64 of each 65-wide head slot): memset the
            # whole tile to 1.0; the projection evicts overwrite cols 0..63.
            v4 = v_sb[:].rearrange("p kt (h c) -> p kt h c", h=4)
            nc.gpsimd.memset(v_sb[:].bitcast(f32), 1.0)

            # ================= era 1: x^T, projections, RoPE =================
            with (
                tc.tile_pool(name="w", bufs=1) as w_pool,
                tc.tile_pool(name="xload", bufs=3) as x_pool,
                tc.tile_pool(name="xT", bufs=2) as xT_pool,
                tc.tile_pool(name="rope", bufs=2) as rope_pool,
                tc.tile_pool(name="ps_tr", bufs=2, space="PSUM") as ps_tr,
                tc.tile_pool(name="ps_proj", bufs=4, space="PSUM") as ps_proj,
                tc.tile_pool(name="ps_v", bufs=2, space="PSUM") as ps_v,
            ):
                w_sb = w_pool.tile([128, NKC, 768], f32r, tag="w")
                ident = w_pool.tile([128, 128], f32, tag="ident")
                make_identity(nc, ident[:])

                stage = {}
                for qtr in range(NQTR):
                    # ---- x^T for this 512-row slab of x
                    xT_q = xT_pool.tile([128, NKC, 512], f32r, tag="xTq")
                    for tl in range(4):
                        xt = x_pool.tile([128, C], f32, tag="x")
                        t0 = qtr * 512 + tl * 128
                        nc.scalar.dma_start(xt[:], x_d[t0:t0 + 128, :])
                        for kc2 in range(NKC // 2):
                            pt = ps_tr.tile([128, 2, 128], f32, tag="tr")
                            for j in range(2):
                                kc = 2 * kc2 + j
                                nc.tensor.transpose(pt[:, j, :], xt[:, kc * 128:(kc + 1) * 128], ident[:])
                            nc.scalar.copy(xT_q[:, 2 * kc2:2 * kc2 + 2, tl * 128:(tl + 1) * 128], pt[:])

                    if qtr == 0:
                        # big weight/table loads issued after quarter-0 x tiles,
                        # so the x DMAs get the full HBM bandwidth at startup
                        nc.sync.dma_start(w_sb[:], w_d[:].rearrange("(kc p) f -> p kc f", p=128).bitcast(f32r))
                        nc.sync.dma_start(cs_sb[:], cs_d[:])
                        nc.sync.dma_start(sn_sb[:], sn_d[:])

                    # ---- QK projection + RoPE (pairs: (QE,QO) then (KE,KO))
                    cs_c = cs_sb[:, qtr * 512:(qtr + 1) * 512]
                    sn_c = sn_sb[:, qtr * 512:(qtr + 1) * 512]
                    for pair in range(2):          # 0: Q, 1: K
                        m_e, m_o = 2 * pair, 2 * pair + 1
                        ps_e = ps_proj.tile([128, 512], f32, tag="proj")
                        ps_o = ps_proj.tile([128, 512], f32, tag="proj")
                        for kc in range(NKC):
                            nc.tensor.matmul(ps_e[:], w_sb[:, kc, m_e * 128:(m_e + 1) * 128],
                                             xT_q[:, kc, :], start=(kc == 0), stop=(kc == NKC - 1))
                            nc.tensor.matmul(ps_o[:], w_sb[:, kc, m_o * 128:(m_o + 1) * 128],
                                             xT_q[:, kc, :], start=(kc == 0), stop=(kc == NKC - 1))
                        t1 = rope_pool.tile([128, 512], f32, tag="t1")
                        t2 = rope_pool.tile([128, 512], f32, tag="t2")
                        if qtr % 2 == 0:
                            stage[pair] = (
                                rope_pool.tile([128, 1024], f32r, tag=f"ev{pair}", name=f"ev{pair}"),
                                rope_pool.tile([128, 1024], f32r, tag=f"od{pair}", name=f"od{pair}"),
                            )
                        evst, odst = stage[pair]
                        hb = (qtr % 2) * 512
                        ev = evst[:, hb:hb + 512]
                        od = odst[:, hb:hb + 512]
                        nc.vector.tensor_tensor(t1[:], ps_e[:], cs_c, MUL)
                        nc.vector.tensor_tensor(t2[:], ps_o[:], sn_c, MUL)
                        nc.vector.tensor_tensor(ev, t1[:], t2[:], SUB)
                        nc.vector.tensor_tensor(t1[:], ps_e[:], sn_c, MUL)
                        nc.vector.tensor_tensor(t2[:], ps_o[:], cs_c, MUL)
                        nc.vector.tensor_tensor(od, t1[:], t2[:], ADD)
                        # merge into per-head-contiguous tiles every 2 quarters
                        if qtr % 2 == 1:
                            dstT = qT if pair == 0 else kT
                            sl = slice((qtr - 1) * 512, (qtr + 1) * 512)
                            for h in range(4):
                                h2, hh = h // 2, h % 2
                                r0 = hh * 64
                                nc.sync.dma_start(dstT[h2][r0:r0 + 32, sl], evst[h * 32:(h + 1) * 32, :])
                                nc.sync.dma_start(dstT[h2][r0 + 32:r0 + 64, sl], odst[h * 32:(h + 1) * 32, :])

                    # ---- V projection (t-on-partition layout)
                    for tl in range(4):
                        psv = ps_v.tile([128, 256], f32, tag="v")
                        for kc in range(NKC):
                            nc.tensor.matmul(psv[:], xT_q[:, kc, tl * 128:(tl + 1) * 128],
                                             w_sb[:, kc, 512:768], start=(kc == 0), stop=(kc == NKC - 1))
                        kt = qtr * 4 + tl
                        nc.scalar.copy(v4[:, kt, :, 0:64], psv[:].rearrange("p (h d) -> p h d", h=4))

            # ============ era 2: attention + per-chunk out-projection ============
            if 2 not in phases:
                # timing bisect: stop after era 1
                pass
            else:
              with (
                  tc.tile_pool(name="pt", bufs=4) as pt_pool,
                  tc.tile_pool(name="nrm", bufs=2) as nrm_pool,
                  tc.tile_pool(name="yout", bufs=3) as y_pool,
                  tc.tile_pool(name="ps_s", bufs=2, space="PSUM") as ps_s_pool,
                  tc.tile_pool(name="ps_o", bufs=1, space="PSUM") as ps_o_pool,
              ):
                  # wo/tri needed from here on
                  nc.sync.dma_start(wo_sb[:], wo_d[:].rearrange("(kc p) c -> p kc c", p=128).bitcast(f32r))
                  nc.sync.dma_start(tri_sb[:], tri_d[:])

                  for qc in range(NQTR):
                      nkt_q = (qc + 1) * 4
                      for hp in range(2):          # head pair (tile index)
                          ps_o = [ps_o_pool.tile([65, 512], f32, tag=f"o{hh}", name=f"pso{hh}") for hh in range(2)]
                          for ki2 in range(nkt_q // 2):
                              ki0 = 2 * ki2
                              # S^T for BOTH heads first, interleaving the T0/T8
                              # row-tiles so both halves of the PE array overlap
                              ps_s2 = [ps_s_pool.tile([128, 1024], f32, tag="s", name=f"ps_s{hh}")
                                       for hh in range(2)]
                              for half in range(2):
                                  ki = ki0 + half
                                  soff = max(0, ki * 128 - qc * 512) if ki // 4 == qc else 0
                                  for hh in range(2):
                                      r0 = hh * 64
                                      nc.tensor.matmul(
                                          ps_s2[hh][:, half * 512 + soff:(half + 1) * 512],
                                          kT[hp][r0:r0 + 64, ki * 128:(ki + 1) * 128],
                                          qT[hp][r0:r0 + 64, qc * 512 + soff:(qc + 1) * 512],
                                          start=True, stop=True, skip_group_check=True)
                              pts = []
                              for hh in range(2):
                                  ps_s = ps_s2[hh]
                                  pt = pt_pool.tile([128, 1024], f32r, tag="pt", name=f"pt{hh}")
                                  pts.append(pt)
                                  if ki0 // 4 != qc and (ki0 + 1) // 4 != qc:
                                      # both tiles fully below the diagonal band
                                      nc.scalar.activation(pt[:], ps_s[:], Exp, scale=0.125)
                                  else:
                                      for half in range(2):
                                          ki = ki0 + half
                                          base = half * 512
                                          off = ki * 128 - qc * 512
                                          if ki // 4 == qc:   # diagonal tile
                                              if off > 0:
                                                  nc.gpsimd.memset(pt[:, base:base + off].bitcast(f32), 0.0)
                                              nc.scalar.activation(pt[:, base + off:base + 512],
                                                                   ps_s[:, base + off:base + 512], Exp, scale=0.125)
                                              nc.vector.tensor_tensor(
                                                  pt[:, base + off:base + off + 128],
                                                  pt[:, base + off:base + off + 128].bitcast(f32),
                                                  tri_sb[:], MUL)
                                          else:
                                              nc.scalar.activation(pt[:, base:base + 512],
                                                                   ps_s[:, base:base + 512], Exp, scale=0.125)
                              for hh in range(2):
                                  h = hp * 2 + hh
                                  for half in range(2):
                                      ki = ki0 + half
                                      soff = max(0, ki * 128 - qc * 512) if (ki // 4 == qc and ki != 0) else 0
                                      nc.tensor.matmul(ps_o[hh][:, soff:512],
                                                       v_sb[:, ki, h * 65:(h + 1) * 65],
                                                       pts[hh][:, half * 512 + soff:(half + 1) * 512],
                                                       start=(ki == 0), stop=(ki == nkt_q - 1),
                                                       skip_group_check=True)
                          # normalize and write A^T rows
                          for hh in range(2):
                              rrow = nrm_pool.tile([1, 512], f32, tag="rrow")
                              nc.vector.reciprocal(rrow[:], ps_o[hh][64:65, :])
                              bsum = nrm_pool.tile([64, 512], f32, tag="bsum")
                              nc.sync.dma_start(bsum[:], rrow[0:1, None, :].to_broadcast([1, 64, 512]))
                              nc.vector.tensor_tensor(
                                  a_sb[hp][hh * 64:(hh + 1) * 64, qc * 512:(qc + 1) * 512],
                                  ps_o[hh][0:64, :], bsum[:], MUL)

                      # ---- out-projection for this query chunk
                      for tl in range(4):
                          ti = qc * 4 + tl
                          for ncol in range(2):
                              psy = ps_s_pool.tile([128, 512], f32, tag="y")
                              for kc2 in range(2):
                                  nc.tensor.matmul(psy[:], a_sb[kc2][:, ti * 128:(ti + 1) * 128],
                                                   wo_sb[:, kc2, ncol * 512:(ncol + 1) * 512],
                                                   start=(kc2 == 0), stop=(kc2 == 1))
                              yt = y_pool.tile([128, 512], f32, tag="yt")
                              nc.vector.tensor_copy(yt[:], psy[:])
                              nc.scalar.dma_start(y_d[ti * 128:(ti + 1) * 128, ncol * 512:(ncol + 1) * 512], yt[:])

    if split_waits:
        _split_multi_waits(nc)
    return nc


# ---------------------------------------------------------------------------
# Host-side sharding / gather
# ---------------------------------------------------------------------------

def _rope_tables():
    inv_freq = (1.0 / (ROPE_BASE ** (np.arange(0, D, 2, dtype=np.float32) / D))).astype(np.float32)
    ang = (np.arange(T, dtype=np.float32)[:, None] * inv_freq[None, :]).astype(np.float32)  # (T, 32)
    cos, sin = np.cos(ang), np.sin(ang)
    idx = np.arange(128) % 32
    return np.ascontiguousarray(cos[:, idx].T), np.ascontiguousarray(sin[:, idx].T)  # (128, T)


def _perm_cols(g):
    """w_qkv column order for core group g: [QE|QO|KE|KO|V]."""
    cols = []
    for base, par in ((0, 0), (0, 1), (C, 0), (C, 1)):      # QE, QO, KE, KO
        for hl in range(4):
            hg = g * 4 + hl
            for i in range(32):
                cols.append(base + hg * 64 + 2 * i + par)
    for hl in range(4):
        hg = g * 4 + hl
        for d_ in range(64):
            cols.append(2 * C + hg * 64 + d_)
    return np.asarray(cols)


def _perm_wo_rows(g):
    # O^T rows follow v's ORIGINAL dim order (RoPE only permutes q/k dims),
    # so the out-projection rows are just this head group's contiguous block.
    return np.arange(g * 256, (g + 1) * 256)


def make_in_maps(x, w_qkv, w_out):
    x = np.asarray(x, np.float32)
    w_qkv = np.asarray(w_qkv, np.float32)
    w_out = np.asarray(w_out, np.float32)
    cs, sn = _rope_tables()
    tri = np.tril(np.ones((128, 128), np.float32)).T  # tri[k, q] = 1 iff q >= k
    in_maps = []
    for c in range(NCORES):
        b, g = c // 4, c % 4
        in_maps.append({
            "x": np.ascontiguousarray(x[b]),
            "w": np.ascontiguousarray(w_qkv[:, _perm_cols(g)]),
            "wo": np.ascontiguousarray(w_out[_perm_wo_rows(g), :]),
            "cs": cs, "sn": sn, "tri": tri,
        })
    return in_maps


def kernel(x, w_qkv, w_out):
    global _BUILT
    from concourse.bass_utils import run_bass_kernel_spmd

    if _BUILT is None:
        _BUILT = build_nc()
    in_maps = make_in_maps(x, w_qkv, w_out)
    res = run_bass_kernel_spmd(_BUILT, in_maps, core_ids=list(range(NCORES)))
    out = np.zeros((B, T, C), np.float32)
    for c in range(NCORES):
        out[c // 4] += res.results[c]["y"]
    return out



# revision 31
# speedup vs baseline: 75.8005x; 75.8005x over previous
"""Multi-head self-attention (B=2, T=2048, C=1024, H=16, RoPE, causal) on 8 trn2 cores.

Sharding: data-parallel over batch (2) x tensor-parallel over head groups (4).
Core c handles batch c//4, heads (c%4)*4 .. +3.  Each core computes its
4 heads' attention output and a partial out-projection (contraction over its
256 head-dims); the host sums the 4 partials per batch.

Layout strategy per core:
  - x^T tiles (C on partitions) built via PE transposes.
  - QKV projection emits q^T/k^T with per-head [evens(32); odds(32)] dim
    order (host-permuted w columns), so RoPE is full-width DVE math in a
    [QE_all(128); QO_all(128)] layout, then SBUF->SBUF DMAs merge into
    per-head-contiguous q^T/k^T tiles (two heads per 128-partition tile).
  - Scores computed transposed (S^T[k, q]) so no P-transposes are needed:
    softmax denominator comes free from a ones-column in the O^T stationary.
  - fp32r matmuls throughout (full-speed fp32 mode on trn2).
  - Era 2 is software-pipelined over flattened (qc, hp, head, kpair) items:
    S(i+1) issues ahead of exp(i)/AV(i) so the PE queue never blocks behind
    the Act-engine exp stream; normalize/out-projection work is drip-fed
    between items.  The q/k head-merge runs as PE permutation matmuls
    (PSUM-accumulated ev+od) instead of 8MB/iter of SBUF->SBUF DMA, and DMA
    issue queues are balanced across SP/Act so neither serializes the x
    prefetch.
"""
import sys
import math

sys.path.insert(0, "/opt/trn_rl_repo")

import numpy as np

B, T, C, H, D = 2, 2048, 1024, 16, 64
HG = H // 4            # 4 heads per core
NCORES = 8
NKC = C // 128         # 8 contraction chunks
NQTR = T // 512        # 4 t-quarters
NKT = T // 128         # 16 k-tiles
ROPE_BASE = 10000.0

_BUILT = None


# ---------------------------------------------------------------------------
# Toolchain workaround: this walrus build accepts at most ONE semaphore wait
# per instruction.  Tile's exit drain carries one wait per outstanding proc,
# and stage-1B can attach several waits to compute/DMA instructions.  We
# (a) replace the exit drain with a chain of single-wait drains, and
# (b) post-process the module, hoisting extra waits onto same-engine nops.
# ---------------------------------------------------------------------------

def _apply_tile_patch():
    import bass_rust
    import concourse.tile as tile
    from concourse.vector_clock import ScopedClock

    def _patched_drain_and_barrier(self, tick_clock, wait_clock):
        nc = self.nc
        probe = nc.sync.drain()
        wait_clock.add_sem_waits(probe.ins, ScopedClock({None: tick_clock.global_clock}))
        si = probe.ins.sync_info
        waits = list(si.on_wait) if si is not None else []
        probe.ins.sync_info = None
        name2sem = {s.name: s for s in wait_clock.sems.allocated().values()}
        for w in waits:
            d = nc.sync.drain()
            bass_rust.wait_op(d.ins, name2sem[w.ant_name], w.wait_value, "sem-ge", False)
        nc.all_engine_barrier()
        popped = nc._tile_sem_poison_stack.pop()
        assert popped is self._sem_poison
        nc.clear_and_free_semaphores(list(self.sems.allocated().values()))
        nc.all_engine_barrier()

    tile.TileContext._drain_and_barrier = _patched_drain_and_barrier


def _split_multi_waits(nc):
    import bass_rust
    import concourse.mybir as mybir

    ctr = 0
    for fn in nc.m.functions:
        for blk in fn.blocks:
            il = blk.instructions
            new = []
            changed = False
            for inst in il:
                si = inst.sync_info
                waits = list(si.on_wait) if si is not None else []
                if len(waits) > 1:
                    changed = True
                    for w in waits[:-1]:
                        nop = mybir.InstNoOp(name=f"I-waitsplit-{ctr}", ins=[], outs=[])
                        ctr += 1
                        nop.engine = inst.engine
                        nop.sync_info = bass_rust.SyncInfo(on_wait=[w], on_update=[])
                        new.append(nop)
                    inst.sync_info = bass_rust.SyncInfo(
                        on_wait=[waits[-1]], on_update=list(si.on_update)
                    )
                new.append(inst)
            if changed:
                blk.instructions = new


# ---------------------------------------------------------------------------
# Kernel builder (per-core program; identical on all 8 cores)
# ---------------------------------------------------------------------------

def build_nc(split_waits=True, loop_iters=None, phases=(1, 2)):
    _apply_tile_patch()
    import concourse.bass as bass
    import concourse.tile as tile
    import concourse.mybir as mybir
    from concourse.masks import make_identity
    from contextlib import nullcontext

    dt = mybir.dt
    f32, f32r = dt.float32, dt.float32r
    Exp = mybir.ActivationFunctionType.Exp
    MUL, SUB, ADD, DIV = (mybir.AluOpType.mult, mybir.AluOpType.subtract,
                          mybir.AluOpType.add, mybir.AluOpType.divide)

    nc = bass.Bass()
    x_d = nc.dram_tensor("x", [T, C], f32, kind="ExternalInput")
    w_d = nc.dram_tensor("w", [C, 768], f32, kind="ExternalInput")
    wo_d = nc.dram_tensor("wo", [256, C], f32, kind="ExternalInput")
    cs_d = nc.dram_tensor("cs", [128, T], f32, kind="ExternalInput")
    sn_d = nc.dram_tensor("sn", [128, T], f32, kind="ExternalInput")
    tri_d = nc.dram_tensor("tri", [128, 128], f32, kind="ExternalInput")
    y_d = nc.dram_tensor("y", [T, C], f32, kind="ExternalOutput")

    with tile.TileContext(nc) as tc:
      loop_cm = (tc.For_i(0, loop_iters, 1,
                          hint_engines=(mybir.EngineType.PE, mybir.EngineType.Activation,
                                        mybir.EngineType.DVE, mybir.EngineType.SP,
                                        mybir.EngineType.Pool))
                 if loop_iters else nullcontext())
      with (
          # ---- persistent pools (live across the whole loop; allocated,
          # memset and identity-initialized ONCE, not per iteration)
          tc.tile_pool(name="persist", bufs=1) as persist,
          tc.tile_pool(name="qkT", bufs=1) as qkT_pool,
          tc.tile_pool(name="asb", bufs=1) as asb_pool,
      ):
        # per-head-contiguous rotated q^T/k^T: tile [128, T] = 2 heads
        qT = [qkT_pool.tile([128, T], f32r, tag=f"qT{i}", name=f"qT{i}") for i in range(2)]
        kT = [qkT_pool.tile([128, T], f32r, tag=f"kT{i}", name=f"kT{i}") for i in range(2)]
        # v in (t, d) layout + ones column per head slot: [128, kt, 4*65]
        v_sb = persist.tile([128, NKT, 4 * 65], f32r, tag="v")
        wo_sb = persist.tile([128, 2, C], f32r, tag="wo")
        tri_sb = persist.tile([128, 128], f32, tag="tri")
        cs_sb = persist.tile([128, T], f32, tag="cs")
        sn_sb = persist.tile([128, T], f32, tag="sn")
        a_sb = [asb_pool.tile([128, T], f32r, tag=f"a{i}", name=f"a{i}") for i in range(2)]
        ident = persist.tile([128, 128], f32, tag="ident")
        make_identity(nc, ident[:])
        # merge permutation stationaries: dst-tile h2 rows h_l*64+j (evens)
        # and h_l*64+32+j (odds) take stage rows (2*h2+h_l)*32+j.  Built once
        # from identity columns; Act copies round them into f32r.
        pev = persist.tile([128, 2, 128], f32r, tag="pev")
        pod = persist.tile([128, 2, 128], f32r, tag="pod")
        nc.gpsimd.memset(pev[:].bitcast(f32), 0.0)
        nc.gpsimd.memset(pod[:].bitcast(f32), 0.0)
        for h2 in range(2):
            for hl in range(2):
                src_cols = ident[:, (2 * h2 + hl) * 32:(2 * h2 + hl) * 32 + 32]
                nc.scalar.copy(pev[:, h2, hl * 64:hl * 64 + 32], src_cols)
                nc.scalar.copy(pod[:, h2, hl * 64 + 32:hl * 64 + 64], src_cols)

        # ones columns of v (col 64 of each 65-wide head slot): memset the
        # whole tile to 1.0; the projection evicts overwrite cols 0..63 only,
        # so a single pre-loop memset survives every iteration.
        v4 = v_sb[:].rearrange("p kt (h c) -> p kt h c", h=4)
        nc.gpsimd.memset(v_sb[:].bitcast(f32), 1.0)

        with loop_cm:
          if True:

            # ================= era 1: x^T, projections, RoPE =================
            # kc-major per quarter: transposes for contraction chunk kc are
            # immediately followed by that chunk's QK + V accumulation
            # matmuls, so the projections start ~1/8 of a quarter after the
            # first x tile lands instead of after the whole slab transposes.
            with (
                tc.tile_pool(name="w", bufs=1) as w_pool,
                tc.tile_pool(name="xload", bufs=1) as x_pool,
                tc.tile_pool(name="xT", bufs=2) as xT_pool,
                tc.tile_pool(name="rope", bufs=2) as rope_pool,
                tc.tile_pool(name="ps_tr", bufs=2, space="PSUM") as ps_tr,
                tc.tile_pool(name="ps_proj", bufs=4, space="PSUM") as ps_proj,
                tc.tile_pool(name="ps_v", bufs=1, space="PSUM") as ps_v,
            ):
                w_sb = w_pool.tile([128, NKC, 768], f32r, tag="w")
                w_r = w_d[:].rearrange("(kc p) f -> p kc f", p=128).bitcast(f32r)

                stage = {}
                deferred_merges = []
                for qtr in range(NQTR):
                    xts = []
                    for tl in range(4):
                        xt = x_pool.tile([128, C], f32, tag="x", bufs=4, name="xt")
                        t0 = qtr * 512 + tl * 128
                        # alternate DMA issue queues so the x stream is not
                        # serialized behind one DGE queue
                        (nc.sync if tl % 2 == 0 else nc.scalar).dma_start(xt[:], x_d[t0:t0 + 128, :])
                        xts.append(xt)
                    while deferred_merges:
                        deferred_merges.pop(0)()

                    if qtr == 0:
                        # per-chunk weight loads so QK(kc=0) waits only 1/8 of
                        # w; on the Act queue to keep the x stream unblocked
                        for kc in range(NKC):
                            nc.sync.dma_start(w_sb[:, kc, :], w_r[:, kc, :])
                        nc.sync.dma_start(cs_sb[:], cs_d[:])
                        nc.sync.dma_start(sn_sb[:], sn_d[:])

                    xT_q = xT_pool.tile([128, NKC, 512], f32r, tag="xTq")
                    ps_q = [ps_proj.tile([128, 512], f32, tag="proj", name=f"psq{m}")
                            for m in range(4)]       # QE, QO, KE, KO
                    psv4 = ps_v.tile([128, 4, 256], f32, tag="v")
                    def proj_chunk(kc):
                        for m in range(4):
                            nc.tensor.matmul(ps_q[m][:], w_sb[:, kc, m * 128:(m + 1) * 128],
                                             xT_q[:, kc, :], start=(kc == 0), stop=(kc == NKC - 1),
                                             skip_group_check=True)

                    # All transposes first: they give the PE a ~3us buffer at
                    # the quarter head while QK(kc=0) waits for the ps_q ring
                    # (previous quarter's RoPE reads) and the kc=0 eviction.
                    for kc in range(NKC):
                        pt = ps_tr.tile([128, 4, 128], f32, tag="tr")
                        for tl in range(4):
                            nc.tensor.transpose(pt[:, tl, :], xts[tl][:, kc * 128:(kc + 1) * 128], ident[:])
                        # alternate evictions Act/DVE: either engine alone is
                        # slower than the PE transposes and would rate-limit
                        if kc % 2 == 0:
                            nc.scalar.copy(xT_q[:, kc, :], pt[:])
                        else:
                            nc.vector.tensor_copy(xT_q[:, kc, :], pt[:])
                    for kc in range(NKC):
                        proj_chunk(kc)
                    # V projection: accumulation groups must stay contiguous
                    # per tl slice — two concurrently-open matmul accumulation
                    # groups in one PSUM bank corrupt each other
                    for tl in range(4):
                        for kc in range(NKC):
                            nc.tensor.matmul(psv4[:, tl, :], xT_q[:, kc, tl * 128:(tl + 1) * 128],
                                             w_sb[:, kc, 512:768], start=(kc == 0), stop=(kc == NKC - 1),
                                             skip_group_check=True)

                    # ---- RoPE (pairs: (QE,QO) then (KE,KO))
                    cs_c = cs_sb[:, qtr * 512:(qtr + 1) * 512]
                    sn_c = sn_sb[:, qtr * 512:(qtr + 1) * 512]
                    for pair in range(2):          # 0: Q, 1: K
                        ps_e, ps_o = ps_q[2 * pair], ps_q[2 * pair + 1]
                        t1 = rope_pool.tile([128, 512], f32, tag="ta", name="t1")
                        t2 = rope_pool.tile([128, 512], f32, tag="tb", name="t2")
                        t3 = rope_pool.tile([128, 512], f32, tag="ta", name="t3")
                        t4 = rope_pool.tile([128, 512], f32, tag="tb", name="t4")
                        if qtr % 2 == 0:
                            stage[pair] = (
                                rope_pool.tile([128, 1024], f32r, tag=f"ev{pair}", name=f"ev{pair}"),
                                rope_pool.tile([128, 1024], f32r, tag=f"od{pair}", name=f"od{pair}"),
                            )
                        evst, odst = stage[pair]
                        hb = (qtr % 2) * 512
                        ev = evst[:, hb:hb + 512]
                        od = odst[:, hb:hb + 512]
                        # products on DVE (PSUM reads); combines on gpsimd
                        # (SBUF-only) to split the RoPE load across engines
                        nc.vector.tensor_tensor(t1[:], ps_e[:], cs_c, MUL)
                        nc.vector.tensor_tensor(t2[:], ps_o[:], sn_c, MUL)
                        nc.vector.tensor_tensor(t3[:], ps_e[:], sn_c, MUL)
                        nc.vector.tensor_tensor(t4[:], ps_o[:], cs_c, MUL)
                        nc.gpsimd.tensor_tensor(ev, t1[:], t2[:], SUB)
                        nc.gpsimd.tensor_tensor(od, t3[:], t4[:], ADD)
                        # merge into per-head-contiguous tiles every 2
                        # quarters — via PE permutation matmuls (PSUM
                        # accumulate folds ev+od), keeping 8MB/iter of
                        # SBUF->SBUF traffic off the DMA queues.  Deferred to
                        # after the NEXT quarter's x-load issues.
                        if qtr % 2 == 1:
                            def _merge(pair=pair, evst=evst, odst=odst, qtr=qtr):
                                dstT = qT if pair == 0 else kT
                                c0 = (qtr - 1) * 512
                                for h2 in range(2):
                                    for blk in range(2):
                                        mps = ps_tr.tile([128, 512], f32, tag="tr", name="mps")
                                        nc.tensor.matmul(mps[:], pev[:, h2, :],
                                                         evst[:, blk * 512:(blk + 1) * 512],
                                                         start=True, stop=False, skip_group_check=True)
                                        nc.tensor.matmul(mps[:], pod[:, h2, :],
                                                         odst[:, blk * 512:(blk + 1) * 512],
                                                         start=False, stop=True, skip_group_check=True)
                                        dst = dstT[h2][:, c0 + blk * 512:c0 + (blk + 1) * 512]
                                        if (h2 + blk) % 2 == 0:
                                            nc.scalar.copy(dst, mps[:])
                                        else:
                                            nc.vector.tensor_copy(dst, mps[:])
                            deferred_merges.append(_merge)

                    # ---- V eviction (t-on-partition layout), one wide copy
                    kt0 = qtr * 4
                    nc.scalar.copy(v4[:, kt0:kt0 + 4, :, 0:64],
                                   psv4[:].rearrange("p tl (h d) -> p tl h d", h=4))

                while deferred_merges:
                    deferred_merges.pop(0)()

            # ============ era 2: attention + per-chunk out-projection ============
            # Software-pipelined: flattened items (qc, hp, hh, kpair); issue
            # order skews S(i+1) ahead of exp(i)/AV(i) so the PE never sits
            # behind the Act-engine exp in its own queue.  exp runs full-width
            # [128,1024]; masked diagonal-band columns are zeroed AFTER on
            # gpsimd (cheaper than splitting the activation per sub-range).
            if 2 not in phases:
                # timing bisect: stop after era 1
                pass
            else:
              with (
                  tc.tile_pool(name="pt", bufs=4) as pt_pool,
                  tc.tile_pool(name="nrm", bufs=2) as nrm_pool,
                  tc.tile_pool(name="yout", bufs=3) as y_pool,
                  tc.tile_pool(name="ps_s", bufs=2, space="PSUM") as ps_s_pool,
                  tc.tile_pool(name="ps_o", bufs=1, space="PSUM") as ps_o_pool,
                  tc.tile_pool(name="ps_y", bufs=2, space="PSUM") as ps_y_pool,
              ):
                  # wo/tri needed from here on
                  nc.sync.dma_start(wo_sb[:], wo_d[:].rearrange("(kc p) c -> p kc c", p=128).bitcast(f32r))
                  nc.sync.dma_start(tri_sb[:], tri_d[:])

                  items = []
                  for qc in range(NQTR):
                      nkt_q = (qc + 1) * 4
                      for hp in range(2):
                          for kp in range(nkt_q // 2):
                              for hh in range(2):
                                  items.append((qc, hp, hh, kp, nkt_q))

                  ps_o_grp = {}   # (qc, hp) -> [ps_o tile per hh]
                  s_tile = {}     # item -> ps_s tile
                  p_tile = {}     # item -> pt tile

                  def do_S(it):
                      qc, hp, hh, kp, nkt_q = it
                      if hh == 0 and kp == 0:
                          ps_o_grp[(qc, hp)] = [
                              ps_o_pool.tile([65, 512], f32, tag=f"o{h}", name=f"pso{h}")
                              for h in range(2)]
                      ps_s = ps_s_pool.tile([128, 1024], f32, tag="s", name="ps_s")
                      s_tile[it] = ps_s
                      r0 = hh * 64
                      for half in range(2):
                          ki = 2 * kp + half
                          # full 512-col write even on diagonal tiles: the
                          # masked columns are finite garbage that exp+memset
                          # discard, and it keeps every PSUM byte the exp
                          # reads defined by this group's own matmuls
                          nc.tensor.matmul(
                              ps_s[:, half * 512:(half + 1) * 512],
                              kT[hp][r0:r0 + 64, ki * 128:(ki + 1) * 128],
                              qT[hp][r0:r0 + 64, qc * 512:(qc + 1) * 512],
                              start=True, stop=True, skip_group_check=True)

                  def do_exp(it):
                      qc, hp, hh, kp, nkt_q = it
                      ps_s = s_tile.pop(it)
                      pt = pt_pool.tile([128, 1024], f32r, tag="pt", name="pt")
                      p_tile[it] = pt
                      offs = []
                      for half in range(2):
                          ki = 2 * kp + half
                          offs.append(ki * 128 - qc * 512 if ki // 4 == qc else 0)
                      if offs[0] + offs[1] >= 512:
                          # mostly masked: exp only the live column ranges
                          for half in range(2):
                              base = half * 512
                              nc.scalar.activation(pt[:, base + offs[half]:base + 512],
                                                   ps_s[:, base + offs[half]:base + 512],
                                                   Exp, scale=0.125)
                      else:
                          nc.scalar.activation(pt[:], ps_s[:], Exp, scale=0.125)
                      for half in range(2):
                          ki = 2 * kp + half
                          if ki // 4 == qc:      # diagonal-quarter tile
                              base = half * 512
                              off = offs[half]
                              if off > 0:
                                  nc.gpsimd.memset(pt[:, base:base + off].bitcast(f32), 0.0)
                              nc.vector.tensor_tensor(
                                  pt[:, base + off:base + off + 128],
                                  pt[:, base + off:base + off + 128].bitcast(f32),
                                  tri_sb[:], MUL)

                  def do_AV(it):
                      qc, hp, hh, kp, nkt_q = it
                      pt = p_tile.pop(it)
                      ps_o = ps_o_grp[(qc, hp)][hh]
                      h = hp * 2 + hh
                      for half in range(2):
                          ki = 2 * kp + half
                          soff = max(0, ki * 128 - qc * 512) if (ki // 4 == qc and ki != 0) else 0
                          nc.tensor.matmul(ps_o[:, soff:512],
                                           v_sb[:, ki, h * 65:(h + 1) * 65],
                                           pt[:, half * 512 + soff:(half + 1) * 512],
                                           start=(ki == 0), stop=(ki == nkt_q - 1),
                                           skip_group_check=True)

                  def finish_hp(qc, hp):
                      ps_o = ps_o_grp.pop((qc, hp))
                      for hh in range(2):
                          rrow = nrm_pool.tile([1, 512], f32, tag="rrow")
                          nc.vector.reciprocal(rrow[:], ps_o[hh][64:65, :])
                          bsum = nrm_pool.tile([64, 512], f32, tag="bsum")
                          nc.sync.dma_start(bsum[:], rrow[0:1, None, :].to_broadcast([1, 64, 512]))
                          nc.vector.tensor_tensor(
                              a_sb[hp][hh * 64:(hh + 1) * 64, qc * 512:(qc + 1) * 512],
                              ps_o[hh][0:64, :], bsum[:], MUL)

                  def do_outproj(qc, tl):
                      ti = qc * 4 + tl
                      for ncol in range(2):
                          psy = ps_y_pool.tile([128, 512], f32, tag="y", name="psy")
                          for kc2 in range(2):
                              nc.tensor.matmul(psy[:], a_sb[kc2][:, ti * 128:(ti + 1) * 128],
                                               wo_sb[:, kc2, ncol * 512:(ncol + 1) * 512],
                                               start=(kc2 == 0), stop=(kc2 == 1),
                                               skip_group_check=True)
                          yt = y_pool.tile([128, 512], f32, tag="yt")
                          nc.vector.tensor_copy(yt[:], psy[:])
                          nc.sync.dma_start(y_d[ti * 128:(ti + 1) * 128, ncol * 512:(ncol + 1) * 512], yt[:])

                  # pending post-actions drain ONE per retired item, spreading
                  # normalize / out-projection work between the S/exp/AV slots
                  # so no engine queue gets a long blocking burst.
                  pending = []
                  nflush = [0]

                  def flush_one(force=False):
                      # normalizes run immediately; out-proj chunks at half
                      # rate so the psy ring never head-blocks the PE queue
                      if pending:
                          nflush[0] += 1
                          if force or pending[0][0] == "norm" or nflush[0] % 2 == 0:
                              kind, fn = pending.pop(0)
                              fn()

                  def retire(it):
                      do_exp(it)
                      do_AV(it)
                      flush_one()
                      qc, hp, hh, kp, nkt_q = it
                      if hh == 1 and kp == nkt_q // 2 - 1:
                          pending.append(("norm", lambda qc=qc, hp=hp: finish_hp(qc, hp)))
                          if hp == 1:
                              for tl in range(4):
                                  pending.append(("proj", lambda qc=qc, tl=tl: do_outproj(qc, tl)))

                  prev = None
                  for it in items:
                      do_S(it)
                      if prev is not None:
                          retire(prev)
                      prev = it
                  retire(prev)
                  while pending:
                      flush_one(force=True)

    if split_waits:
        _split_multi_waits(nc)
    return nc


# ---------------------------------------------------------------------------
# Host-side sharding / gather
# ---------------------------------------------------------------------------

def _rope_tables():
    inv_freq = (1.0 / (ROPE_BASE ** (np.arange(0, D, 2, dtype=np.float32) / D))).astype(np.float32)
    ang = (np.arange(T, dtype=np.float32)[:, None] * inv_freq[None, :]).astype(np.float32)  # (T, 32)
    cos, sin = np.cos(ang), np.sin(ang)
    idx = np.arange(128) % 32
    return np.ascontiguousarray(cos[:, idx].T), np.ascontiguousarray(sin[:, idx].T)  # (128, T)


def _perm_cols(g):
    """w_qkv column order for core group g: [QE|QO|KE|KO|V]."""
    cols = []
    for base, par in ((0, 0), (0, 1), (C, 0), (C, 1)):      # QE, QO, KE, KO
        for hl in range(4):
            hg = g * 4 + hl
            for i in range(32):
                cols.append(base + hg * 64 + 2 * i + par)
    for hl in range(4):
        hg = g * 4 + hl
        for d_ in range(64):
            cols.append(2 * C + hg * 64 + d_)
    return np.asarray(cols)


def _perm_wo_rows(g):
    # O^T rows follow v's ORIGINAL dim order (RoPE only permutes q/k dims),
    # so the out-projection rows are just this head group's contiguous block.
    return np.arange(g * 256, (g + 1) * 256)


def make_in_maps(x, w_qkv, w_out):
    x = np.asarray(x, np.float32)
    w_qkv = np.asarray(w_qkv, np.float32)
    w_out = np.asarray(w_out, np.float32)
    cs, sn = _rope_tables()
    tri = np.tril(np.ones((128, 128), np.float32)).T  # tri[k, q] = 1 iff q >= k
    in_maps = []
    for c in range(NCORES):
        b, g = c // 4, c % 4
        in_maps.append({
            "x": np.ascontiguousarray(x[b]),
            "w": np.ascontiguousarray(w_qkv[:, _perm_cols(g)]),
            "wo": np.ascontiguousarray(w_out[_perm_wo_rows(g), :]),
            "cs": cs, "sn": sn, "tri": tri,
        })
    return in_maps


def kernel(x, w_qkv, w_out):
    global _BUILT
    from concourse.bass_utils import run_bass_kernel_spmd

    if _BUILT is None:
        _BUILT = build_nc()
    in_maps = make_in_maps(x, w_qkv, w_out)
    res = run_bass_kernel_spmd(_BUILT, in_maps, core_ids=list(range(NCORES)))
    out = np.zeros((B, T, C), np.float32)
    for c in range(NCORES):
        out[c // 4] += res.results[c]["y"]
    return out

